# revision 1
# baseline (speedup 1.0000x reference)
"""Trainium2 Bass kernel for batched multi-head attention.

Problem: N=8, S=1024, E=1024, H=16, DK=64 MultiHeadAttention with a boolean
attention mask, fp32 reference.

Strategy: pure batch data-parallelism -- one batch element per NeuronCore
(8 cores), weights replicated, no collectives.  Per core everything is
computed in a transposed layout so no on-chip transposes are needed:

  xT [E, S] (host-transposed)  --Wq/Wk-->  QT, KT [E, S]
  xT                           --Wv---->   V    [S, E]  (head-major, with a
                                                         ones column per head)
  logitsT[k, q] = KT_h^T-slices @ QT_h    (PSUM, fp32 accum)
  Em = exp(logitsT/8) * (1 - maskT)       (ACT exp -> fp16, DVE mask multiply)
  O_h[d|sum, q] = V_aug_h^T @ Em_h        (fp16 matmul; row 64 = softmax sums)
  oT[e', q] = O_h[0:64] * (1/sums)        (DVE; 1/sums broadcast across
                                           partitions via a DRAM bounce)
  out[q, e] = oT^T-slices @ Wo + bo_eff   (bo_eff = bv@Wo + bo folded on host)
"""

import numpy as np
from contextlib import ExitStack

import concourse.bass as bass
import concourse.mybir as mybir
import concourse.tile as tile
from concourse.vector_clock import ScopedClock
from concourse.bass_utils import run_bass_kernel_spmd

F32 = mybir.dt.float32
F32R = mybir.dt.float32r
BF16 = mybir.dt.bfloat16
F16 = mybir.dt.float16
U8 = mybir.dt.uint8
Exp = mybir.ActivationFunctionType.Exp
Ident = mybir.ActivationFunctionType.Identity
Copy = mybir.ActivationFunctionType.Copy
MULT = mybir.AluOpType.mult

N, S, E, H, DK = 8, 1024, 1024, 16, 64
P = 128
NT = E // P
NPAIR = H // 2

# dtype of the big matmul operands (projections and output projection).
# F16: full-rate matmuls with hidden weight loads.  F32R: ~TF32 precision,
# but self-loading weights make each matmul ~25% slower.
MM_DT = F16


# ---------------------------------------------------------------------------
# Workaround: this walrus build supports at most ONE semaphore wait per
# instruction.  Split instructions carrying more waits into NOP(wait) chains
# on the same engine, and do the same for the TileContext tail drain.
# ---------------------------------------------------------------------------
_MAXW = 1
# instruction types whose lowered ISA struct was observed to accept 2 waits
_MAXW2_TYPES = ()
_orig_lower = tile.TileContext._lower_ordered_insts
_tilefix_installed = False


def _split_waits(ordered):
    for _bb, insts in ordered.items():
        out = []
        for inst in insts:
            si = inst.sync_info
            maxw = 2 if type(inst).__name__ in _MAXW2_TYPES else _MAXW
            if si is not None and len(si.on_wait) > maxw:
                waits = list(si.on_wait)
                keep, extra = waits[:maxw], waits[maxw:]
                for i in range(0, len(extra), _MAXW):
                    out.append(
                        mybir.InstNoOp(
                            name=f"{inst.name}-ws{i}",
                            engine=inst.engine,
                            bass_nofuse=True,
                            sync_info=mybir.SyncInfo(
                                on_wait=extra[i : i + _MAXW], on_update=[]
                            ),
                        )
                    )
                inst.sync_info = mybir.SyncInfo(
                    on_wait=keep, on_update=list(si.on_update)
                )
            out.append(inst)
        insts[:] = out


def _patched_lower(self, ordered):
    _split_waits(ordered)
    return _orig_lower(self, ordered)


def _patched_drain_and_barrier(self, tick_clock, wait_clock):
    nc = self.nc
    drain_inst = nc.sync.drain()
    wait_clock.add_sem_waits(
        drain_inst.ins, ScopedClock({None: tick_clock.global_clock})
    )
    si = drain_inst.ins.sync_info
    waits = list(si.on_wait) if si is not None else []
    if len(waits) > _MAXW:
        drain_inst.ins.sync_info = mybir.SyncInfo(on_wait=[], on_update=[])
        for i in range(0, len(waits), _MAXW):
            nop = nc.sync.nop(nofuse=True)
            nop.ins.sync_info = mybir.SyncInfo(
                on_wait=waits[i : i + _MAXW], on_update=[]
            )
    nc.all_engine_barrier()
    popped = nc._tile_sem_poison_stack.pop()
    assert popped is self._sem_poison
    nc.clear_and_free_semaphores(list(self.sems.allocated().values()))
    nc.all_engine_barrier()


def _install_tilefix():
    global _tilefix_installed
    if not _tilefix_installed:
        tile.TileContext._lower_ordered_insts = _patched_lower
        tile.TileContext._drain_and_barrier = _patched_drain_and_barrier
        _tilefix_installed = True


# ---------------------------------------------------------------------------
# Kernel build
# ---------------------------------------------------------------------------
_cached_nc = None


def _build(repeat=1, mm_dt=None, mask_pair=True):
    global _cached_nc
    if _cached_nc is not None and repeat == 1 and mm_dt is None and mask_pair:
        return _cached_nc
    if mm_dt is None:
        mm_dt = MM_DT
    _install_tilefix()

    nc = bass.Bass("TRN2", num_devices=N)

    x_t = nc.declare_dram_parameter("x_t", [E, S], mm_dt, isOutput=False)
    mask_t = nc.declare_dram_parameter("mask_t", [S, S], U8, isOutput=False)
    wq = nc.declare_dram_parameter("wq", [E, E], mm_dt, isOutput=False)
    wk = nc.declare_dram_parameter("wk", [E, E], mm_dt, isOutput=False)
    wv = nc.declare_dram_parameter("wv", [E, E], mm_dt, isOutput=False)
    wo = nc.declare_dram_parameter("wo", [E, E], mm_dt, isOutput=False)
    bqc = nc.declare_dram_parameter("bqc", [P, NT], F32, isOutput=False)
    bkc = nc.declare_dram_parameter("bkc", [P, NT], F32, isOutput=False)
    bo_eff = nc.declare_dram_parameter("bo_eff", [E], F32, isOutput=False)
    out = nc.declare_dram_parameter("out", [S, E], F32, isOutput=True)

    def tiled(ap):
        return ap.rearrange("(t p) f -> p t f", p=P)

    x_tt = tiled(x_t.ap())
    mask_tt = tiled(mask_t.ap())
    w_t = {
        "q": tiled(wq.ap()),
        "k": tiled(wk.ap()),
        "v": tiled(wv.ap()),
        "o": tiled(wo.ap()),
    }
    out_t = tiled(out.ap())

    with tile.TileContext(nc) as tc, ExitStack() as ctx:
        # persistent pools, longest lifetime first (stack allocator)
        p_oT = ctx.enter_context(tc.tile_pool(name="oT", bufs=1))
        p_pers = ctx.enter_context(tc.tile_pool(name="pers", bufs=1))

        oT = p_oT.tile([P, NT, S], mm_dt)
        QT = p_pers.tile([P, NT, S], mm_dt)
        KT = p_pers.tile([P, NT, S], mm_dt)
        Vg = p_pers.tile([P, NT, H * (DK + 1)], F16)
        nm = p_pers.tile([P, NT, S], F16)
        bq_sb = p_pers.tile([P, NT], F32)
        bk_sb = p_pers.tile([P, NT], F32)
        bo_sb = p_pers.tile([P, S], F32)
        p_woful = ctx.enter_context(tc.tile_pool(name="wofull", bufs=1))
        Wof = p_woful.tile([P, NT, S], mm_dt)

        for rep in range(repeat):
            nc.sync.dma_start(bq_sb[:], bqc[:])
            nc.sync.dma_start(bk_sb[:], bkc[:])
            nc.sync.dma_start(
                bo_sb[:],
                bo_eff.ap().rearrange("(o e) -> o e", o=1).broadcast_to((P, S)),
            )
            nc.any.memset(Vg[:, :, DK :: DK + 1], 1.0)
            ones64 = p_pers.tile([1, DK], F32, name=f"ones64_{rep}")
            nc.any.memset(ones64[:], 1.0)

            # ---- phase A: xT load, mask convert, Q/K/V projections ----
            with tc.tile_pool(name="phAw", bufs=1) as p_w, \
                 tc.tile_pool(name="phA", bufs=1) as p_x, \
                 tc.tile_pool(name="phAm", bufs=2) as p_m, \
                 tc.tile_pool(name="psA", bufs=8, space="PSUM") as psA:

                Wf = {
                    pr: p_w.tile([P, NT, S], mm_dt, name=f"Wf_{rep}_{pr}")
                    for pr in ("v", "q", "k")
                }
                xT = p_x.tile([P, NT, S], mm_dt, name=f"xT_{rep}")
                # interleave so the first projection's operands arrive first
                for j in range(NT):
                    nc.sync.dma_start(xT[:, j, :], x_tt[:, j, :])
                    nc.sync.dma_start(Wf["v"][:, j, :], w_t["v"][:, j, :])
                for pr in ("q", "k"):
                    for j in range(NT):
                        nc.sync.dma_start(Wf[pr][:, j, :], w_t[pr][:, j, :])
                for j in range(NT):
                    nc.sync.dma_start(Wof[:, j, :], w_t["o"][:, j, :])

                for j in range(NT):
                    mu = p_m.tile([P, S], U8, tag="mu", name=f"mu_{rep}_{j}")
                    nc.sync.dma_start(mu[:], mask_tt[:, j, :])
                    # notm = 1 - mask  (fp16; exact for 0/1)
                    nc.scalar.activation(
                        nm[:, j, :], mu[:], Ident, bias=1.0, scale=-1.0
                    )

                for proj in ("v", "q", "k"):
                    for half in range(2):
                        accs = {}
                        for j in range(NT):
                            wt = Wf[proj][:, j, :]
                            for ti in range(4):
                                t = half * 4 + ti
                                for c in range(2):
                                    if j == 0:
                                        accs[(t, c)] = psA.tile(
                                            [P, 512], F32, tag="acc",
                                            name=f"acc_{rep}_{proj}_{half}_{t}_{c}",
                                        )
                                    if proj == "v":
                                        lhsT = xT[:, j, t * P : (t + 1) * P]
                                        rhs = wt[:, c * 512 : (c + 1) * 512]
                                    else:
                                        lhsT = wt[:, t * P : (t + 1) * P]
                                        rhs = xT[:, j, c * 512 : (c + 1) * 512]
                                    nc.tensor.matmul(
                                        accs[(t, c)][:], lhsT, rhs,
                                        start=(j == 0), stop=(j == NT - 1),
                                    )
                        for ti in range(4):
                            t = half * 4 + ti
                            for c in range(2):
                                acc = accs[(t, c)]
                                if proj == "q":
                                    nc.vector.tensor_scalar_add(
                                        QT[:, t, c * 512 : (c + 1) * 512],
                                        acc[:], bq_sb[:, t : t + 1],
                                    )
                                elif proj == "k":
                                    nc.vector.tensor_scalar_add(
                                        KT[:, t, c * 512 : (c + 1) * 512],
                                        acc[:], bk_sb[:, t : t + 1],
                                    )
                                else:
                                    dst = Vg[
                                        :, t,
                                        c * 8 * (DK + 1) : (c + 1) * 8 * (DK + 1),
                                    ].rearrange("p (h d) -> p h d", d=DK + 1)[
                                        :, :, 0:DK
                                    ]
                                    src = acc[:].rearrange(
                                        "p (h d) -> p h d", d=DK
                                    )
                                    nc.scalar.activation(dst, src, Copy)

            # ---- phase B: attention per head pair ----
            with tc.tile_pool(name="phBe", bufs=6) as p_em, \
                 tc.tile_pool(name="phBs", bufs=6) as p_os, \
                 tc.tile_pool(name="phBr", bufs=8) as p_r, \
                 tc.tile_pool(name="phBrb", bufs=3) as p_rb, \
                 tc.tile_pool(name="phBd", bufs=4, space="DRAM") as p_dram, \
                 tc.tile_pool(name="psL", bufs=2, space="PSUM") as psL, \
                 tc.tile_pool(name="psO", bufs=2, space="PSUM") as psO:

                for p in range(NPAIR):
                    O = [
                        psO.tile([DK + 1, S], F32, tag="O", name=f"O_{rep}_{p}_{i}")
                        for i in range(2)
                    ]
                    for j in range(NT):
                        L = [
                            psL.tile([P, S], F32, tag="L", name=f"L_{rep}_{p}_{j}_{i}")
                            for i in range(2)
                        ]
                        for c in range(2):
                            for h in range(2):
                                r0, r1 = h * DK, h * DK + DK
                                nc.tensor.matmul(
                                    L[h][:, c * 512 : c * 512 + 512],
                                    KT[r0:r1, p, j * P : (j + 1) * P],
                                    QT[r0:r1, p, c * 512 : (c + 1) * 512],
                                    start=True, stop=True,
                                )
                        if mask_pair:
                            Emp = p_em.tile(
                                [P, 2 * S], F16, tag="Em", name=f"Em_{rep}_{p}_{j}"
                            )
                            Ems = [Emp[:, 0:S], Emp[:, S : 2 * S]]
                            for h in range(2):
                                nc.scalar.activation(
                                    Ems[h], L[h][:], Exp, scale=0.125
                                )
                            nc.vector.tensor_tensor(
                                Emp[:].rearrange("p (h q) -> p h q", h=2),
                                Emp[:].rearrange("p (h q) -> p h q", h=2),
                                nm[:, j : j + 1, :].broadcast_to((P, 2, S)),
                                MULT,
                            )
                        else:
                            Ems = [
                                p_em.tile(
                                    [P, S], F16, tag="Em", name=f"Em_{rep}_{p}_{j}_{i}"
                                )
                                for i in range(2)
                            ]
                            for h in range(2):
                                nc.scalar.activation(
                                    Ems[h][:], L[h][:], Exp, scale=0.125
                                )
                                # plain 2D step-1 operands keep the DVE in 2x mode
                                nc.vector.tensor_tensor(
                                    Ems[h][:], Ems[h][:], nm[:, j, :], MULT
                                )
                        for h in range(2):
                            head = 2 * p + h
                            for c in range(2):
                                nc.tensor.matmul(
                                    O[h][:, c * 512 : (c + 1) * 512],
                                    Vg[:, j, head * (DK + 1) : (head + 1) * (DK + 1)],
                                    Ems[h][:, c * 512 : (c + 1) * 512],
                                    start=(j == 0), stop=(j == NT - 1),
                                )
                    Rb = p_rb.tile([P, S], F32, tag="Rb", name=f"Rb_{rep}_{p}")
                    Ost = p_os.tile([P, S], F32, tag="Ost", name=f"Ost_{rep}_{p}")
                    for h in range(2):
                        # stage O out of PSUM fast so the accumulator banks
                        # free quickly; DVE so ACT stays on the exp stream
                        nc.vector.tensor_copy(
                            Ost[h * DK : (h + 1) * DK, :], O[h][0:DK, :]
                        )
                        R = p_r.tile([1, S], F32, tag="R", name=f"R_{rep}_{p}_{h}")
                        nc.vector.reciprocal(R[:], O[h][DK : DK + 1, :])
                        Rd = p_dram.tile([1, S], F32, tag="Rd", name=f"Rd_{rep}_{p}_{h}")
                        nc.gpsimd.dma_start(Rd[:], R[:])
                        nc.gpsimd.dma_start(
                            Rb[h * DK : (h + 1) * DK, :],
                            Rd[:].broadcast_to((DK, S)),
                        )
                        nc.vector.tensor_tensor(
                            oT[h * DK : (h + 1) * DK, p, :],
                            Ost[h * DK : (h + 1) * DK, :],
                            Rb[h * DK : (h + 1) * DK, :],
                            MULT,
                        )

            # ---- phase C: output projection ----
            with tc.tile_pool(name="phCo", bufs=2) as p_out, \
                 tc.tile_pool(name="psC", bufs=4, space="PSUM") as psC:
                for half in range(2):
                    F = {}
                    for j in range(NT):
                        wt = Wof[:, j, :]
                        for ti in range(4):
                            t = half * 4 + ti
                            if j == 0:
                                F[t] = psC.tile(
                                    [P, S], F32, tag="F", name=f"F_{rep}_{half}_{t}"
                                )
                            for c in range(2):
                                nc.tensor.matmul(
                                    F[t][:, c * 512 : (c + 1) * 512],
                                    oT[:, j, t * P : (t + 1) * P],
                                    wt[:, c * 512 : (c + 1) * 512],
                                    start=(j == 0), stop=(j == NT - 1),
                                )
                    for ti in range(4):
                        t = half * 4 + ti
                        ot = p_out.tile(
                            [P, S], F32, tag="ot", name=f"ot_{rep}_{half}_{ti}"
                        )
                        nc.vector.tensor_add(ot[:], F[t][:], bo_sb[:])
                        nc.sync.dma_start(out_t[:, t, :], ot[:])

    if repeat == 1 and mm_dt == MM_DT:
        _cached_nc = nc
    return nc


# ---------------------------------------------------------------------------
# Entry point
# ---------------------------------------------------------------------------
def _np_dt(mm_dt):
    if mm_dt == F16:
        return np.float16
    return np.float32


def make_in_maps(x, attn_mask, Wq, bq, Wk, bk, Wv, bv, Wo, bo, mm_dt=None):
    if mm_dt is None:
        mm_dt = MM_DT
    ndt = _np_dt(mm_dt)
    bqc = np.ascontiguousarray(np.asarray(bq, np.float32).reshape(NT, P).T)
    bkc = np.ascontiguousarray(np.asarray(bk, np.float32).reshape(NT, P).T)
    bo_eff = (
        np.asarray(bv, np.float64) @ np.asarray(Wo, np.float64)
        + np.asarray(bo, np.float64)
    ).astype(np.float32)
    wqc = np.asarray(Wq, np.float32).astype(ndt)
    wkc = np.asarray(Wk, np.float32).astype(ndt)
    wvc = np.asarray(Wv, np.float32).astype(ndt)
    woc = np.asarray(Wo, np.float32).astype(ndt)
    in_maps = []
    for n in range(N):
        in_maps.append(
            {
                "x_t": np.ascontiguousarray(np.asarray(x[n], np.float32).T).astype(ndt),
                "mask_t": np.ascontiguousarray(np.asarray(attn_mask[n]).T).astype(np.uint8),
                "wq": wqc, "wk": wkc, "wv": wvc, "wo": woc,
                "bqc": bqc, "bkc": bkc, "bo_eff": bo_eff,
            }
        )
    return in_maps


def kernel(x, attn_mask, Wq, bq, Wk, bk, Wv, bv, Wo, bo, **_):
    nc = _build()
    in_maps = make_in_maps(x, attn_mask, Wq, bq, Wk, bk, Wv, bv, Wo, bo)
    res = run_bass_kernel_spmd(nc, in_maps, list(range(N)))
    outs = np.stack([np.asarray(res.results[n]["out"]) for n in range(N)], axis=0)
    return outs.astype(np.float32)



# revision 2
# speedup vs baseline: 1.7454x; 1.7454x over previous
"""Trainium2 Bass kernel for batched multi-head attention.

Problem: N=8, S=1024, E=1024, H=16, DK=64 MultiHeadAttention with a boolean
attention mask, fp32 reference.  One batch element per NeuronCore (8 cores),
weights replicated, no collectives.

Layouts (per core, f16 compute): xT/QT/KT/oT transposed [E,S] tiled
[P, NT, S]; V head-grouped with a ones column per head (softmax sums ride
row 64 of the attn@V psum accumulation); notm = 1-mask prepared on host as
f16 in [k, q] layout.

Schedule: a single persistent Tile scope.  The attention phase
(logits -> exp -> mask -> attn@V per pair/q-half/k-tile) is ACT/DVE-heavy,
so the Q/K/V projection matmuls are emitted as filler thunks pumped into
the PE-idle slots between logits and attn@V; an ensure() mechanism
force-drains the queue so an inline consumer is never emitted before its
queued producer.  Work pipelines across repeat iterations (weights are
DMA'd once; the next rep's projections overlap this rep's output
projection).  PSUM budget: logits 2x[128,1024] double-buffered (4 banks) +
attn@V out 2x[65,512] (2) + projection accumulator 2x[128,512] (2).

Softmax tail: attn@V outputs are staged out of PSUM immediately
(unnormalized) so the accumulator banks free fast; the two sums rows per
pair are ACT-copied to quadrant-aligned partitions of a staging tile, one
batched DVE reciprocal + one DRAM-bounce broadcast per pair produce the
1/sums tiles, and the normalize multiply is deferred one pair so no engine
ever waits on the bounce.  bk is dropped entirely (a per-q constant in the
logits cancels in softmax); bv@Wo+bo is folded on the host.
"""

import os
import numpy as np
from collections import deque
from contextlib import ExitStack

# probe modes for stall attribution: "" | "nomask" | "mask2d"
PROBE = os.environ.get("KPROBE", "")

import concourse.bass as bass
import concourse.mybir as mybir
import concourse.tile as tile
from concourse.vector_clock import ScopedClock
from concourse.bass_utils import run_bass_kernel_spmd

F32 = mybir.dt.float32
F16 = mybir.dt.float16
Exp = mybir.ActivationFunctionType.Exp
Ident = mybir.ActivationFunctionType.Identity
Copy = mybir.ActivationFunctionType.Copy
MULT = mybir.AluOpType.mult

N, S, E, H, DK = 8, 1024, 1024, 16, 64
P = 128
NT = E // P
NPAIR = H // 2
MM_DT = F16

# ---------------------------------------------------------------------------
# Workaround: this walrus build supports at most ONE semaphore wait per
# instruction.  Split instructions carrying more waits into NOP(wait) chains
# on the same engine, and do the same for the TileContext tail drain.
# ---------------------------------------------------------------------------
_MAXW = 1
_orig_lower = tile.TileContext._lower_ordered_insts
_tilefix_installed = False


def _split_waits(ordered):
    for _bb, insts in ordered.items():
        out = []
        for inst in insts:
            si = inst.sync_info
            if si is not None and len(si.on_wait) > _MAXW:
                waits = list(si.on_wait)
                keep, extra = waits[:_MAXW], waits[_MAXW:]
                for i in range(0, len(extra), _MAXW):
                    out.append(
                        mybir.InstNoOp(
                            name=f"{inst.name}-ws{i}",
                            engine=inst.engine,
                            bass_nofuse=True,
                            sync_info=mybir.SyncInfo(
                                on_wait=extra[i : i + _MAXW], on_update=[]
                            ),
                        )
                    )
                inst.sync_info = mybir.SyncInfo(
                    on_wait=keep, on_update=list(si.on_update)
                )
            out.append(inst)
        insts[:] = out


def _patched_lower(self, ordered):
    _split_waits(ordered)
    return _orig_lower(self, ordered)


def _patched_drain_and_barrier(self, tick_clock, wait_clock):
    nc = self.nc
    drain_inst = nc.sync.drain()
    wait_clock.add_sem_waits(
        drain_inst.ins, ScopedClock({None: tick_clock.global_clock})
    )
    si = drain_inst.ins.sync_info
    waits = list(si.on_wait) if si is not None else []
    if len(waits) > _MAXW:
        drain_inst.ins.sync_info = mybir.SyncInfo(on_wait=[], on_update=[])
        for i in range(0, len(waits), _MAXW):
            nop = nc.sync.nop(nofuse=True)
            nop.ins.sync_info = mybir.SyncInfo(
                on_wait=waits[i : i + _MAXW], on_update=[]
            )
    nc.all_engine_barrier()
    popped = nc._tile_sem_poison_stack.pop()
    assert popped is self._sem_poison
    nc.clear_and_free_semaphores(list(self.sems.allocated().values()))
    nc.all_engine_barrier()


def _install_tilefix():
    global _tilefix_installed
    if not _tilefix_installed:
        tile.TileContext._lower_ordered_insts = _patched_lower
        tile.TileContext._drain_and_barrier = _patched_drain_and_barrier
        _tilefix_installed = True


# ---------------------------------------------------------------------------
# Kernel build
# ---------------------------------------------------------------------------
_cached_nc = None


def _build(repeat=1):
    global _cached_nc
    if _cached_nc is not None and repeat == 1:
        return _cached_nc
    _install_tilefix()

    nc = bass.Bass("TRN2", num_devices=N)

    x_t = nc.declare_dram_parameter("x_t", [E, S], MM_DT, isOutput=False)
    notm_t = nc.declare_dram_parameter("notm_t", [S, S], MM_DT, isOutput=False)
    wq = nc.declare_dram_parameter("wq", [E, E], MM_DT, isOutput=False)
    wk = nc.declare_dram_parameter("wk", [E, E], MM_DT, isOutput=False)
    wv = nc.declare_dram_parameter("wv", [E, E], MM_DT, isOutput=False)
    wo = nc.declare_dram_parameter("wo", [E, E], MM_DT, isOutput=False)
    bqc = nc.declare_dram_parameter("bqc", [P, NT], F32, isOutput=False)
    bo_eff = nc.declare_dram_parameter("bo_eff", [E], F32, isOutput=False)
    out = nc.declare_dram_parameter("out", [S, E], F32, isOutput=True)

    def tiled(ap):
        return ap.rearrange("(t p) f -> p t f", p=P)

    x_tt = tiled(x_t.ap())
    notm_tt = tiled(notm_t.ap())
    w_t = {
        "q": tiled(wq.ap()),
        "k": tiled(wk.ap()),
        "v": tiled(wv.ap()),
        "o": tiled(wo.ap()),
    }
    out_t = tiled(out.ap())

    with tile.TileContext(nc) as tc, ExitStack() as ctx:
        # --- persistent pools (allocated once, reused across reps) ---
        p_pers = ctx.enter_context(tc.tile_pool(name="pers", bufs=1))
        p_em = ctx.enter_context(tc.tile_pool(name="em", bufs=4))
        p_sr = ctx.enter_context(tc.tile_pool(name="sr", bufs=2))
        p_rr = ctx.enter_context(tc.tile_pool(name="rr", bufs=2))
        p_rb = ctx.enter_context(tc.tile_pool(name="rb", bufs=4))
        p_ot = ctx.enter_context(tc.tile_pool(name="otst", bufs=2))
        p_dram = ctx.enter_context(tc.tile_pool(name="rdram", bufs=4, space="DRAM"))
        psL = ctx.enter_context(tc.tile_pool(name="psL", bufs=2, space="PSUM"))
        psO = ctx.enter_context(tc.tile_pool(name="psO", bufs=2, space="PSUM"))
        psP = ctx.enter_context(tc.tile_pool(name="psP", bufs=2, space="PSUM"))

        QT = p_pers.tile([P, NT, S], MM_DT)
        KT = p_pers.tile([P, NT, S], MM_DT)
        Vg = p_pers.tile([P, NT, H * (DK + 1)], F16)
        nm = p_pers.tile([P, NT, S], F16)
        oT = p_pers.tile([P, NT, S], MM_DT)
        xT = p_pers.tile([P, NT, S], MM_DT)
        W_sb = {pr: p_pers.tile([P, NT, S], MM_DT, name=f"W_{pr}") for pr in
                ("q", "k", "v", "o")}
        bq_sb = p_pers.tile([P, NT], F32)
        bo_sb = p_pers.tile([P, S], F32)

        # one-time loads / constants
        for j in range(NT):
            for pr in ("q", "k", "v", "o"):
                nc.sync.dma_start(W_sb[pr][:, j, :], w_t[pr][:, j, :])
        nc.sync.dma_start(bq_sb[:], bqc[:])
        nc.sync.dma_start(
            bo_sb[:],
            bo_eff.ap().rearrange("(o e) -> o e", o=1).broadcast_to((P, S)),
        )
        nc.any.memset(Vg[:, :, DK :: DK + 1], 1.0)

        # ---------------- filler thunk machinery ----------------
        # queue items are (key, thunk, is_last_of_key); `ensure` force-
        # drains the queue until a producer group is fully emitted, so an
        # inline consumer can never be emitted before its producer.
        fill = deque()
        emitted = set()

        def pump(n):
            for _ in range(n):
                if not fill:
                    return
                key, th, last = fill.popleft()
                th()
                if last:
                    emitted.add(key)

        def ensure(key):
            while key not in emitted:
                assert fill, f"filler queue empty but {key} not emitted"
                k, th, last = fill.popleft()
                th()
                if last:
                    emitted.add(k)

        def extend_group(key, thunks):
            for i, th in enumerate(thunks):
                fill.append((key, th, i == len(thunks) - 1))

        def qk_group(rep, proj, t, c):
            """Q or K projection for output tile t, q-block c: 8 matmuls
            accumulating in psP + a DVE drain into QT/KT."""
            acc = [None]

            def mk(j):
                def th():
                    if j == 0:
                        acc[0] = psP.tile(
                            [P, 512], F32, tag="pacc",
                            name=f"pacc_{rep}_{proj}_{t}_{c}",
                        )
                    nc.tensor.matmul(
                        acc[0][:],
                        W_sb[proj][:, j, t * P : (t + 1) * P],
                        xT[:, j, c * 512 : (c + 1) * 512],
                        start=(j == 0), stop=(j == NT - 1),
                    )
                return th

            def drain():
                dst = (QT if proj == "q" else KT)[:, t, c * 512 : (c + 1) * 512]
                if proj == "q":
                    nc.vector.tensor_scalar_add(dst, acc[0][:], bq_sb[:, t : t + 1])
                else:
                    nc.vector.tensor_copy(dst, acc[0][:])

            return [mk(j) for j in range(NT)] + [drain]

        def v_group(rep, t, c):
            """V projection for S-tile t, head-half c: 8 matmuls + ACT
            drain into Vg (skipping the ones columns)."""
            acc = [None]

            def mk(j):
                def th():
                    if j == 0:
                        acc[0] = psP.tile(
                            [P, 512], F32, tag="pacc", name=f"vacc_{rep}_{t}_{c}",
                        )
                    nc.tensor.matmul(
                        acc[0][:],
                        xT[:, j, t * P : (t + 1) * P],
                        W_sb["v"][:, j, c * 512 : (c + 1) * 512],
                        start=(j == 0), stop=(j == NT - 1),
                    )
                return th

            def drain():
                dst = Vg[
                    :, t, c * 8 * (DK + 1) : (c + 1) * 8 * (DK + 1)
                ].rearrange("p (h d) -> p h d", d=DK + 1)[:, :, 0:DK]
                src = acc[0][:].rearrange("p (h d) -> p h d", d=DK)
                nc.scalar.activation(dst, src, Copy)

            return [mk(j) for j in range(NT)] + [drain]

        def push_rep_fillers(rep):
            """Queue the projection work of `rep` (V c-half 0 and QK t=0
            first -- needed by pair 0 -- then interleaved in pair order)."""
            for t in range(NT):
                extend_group(("v", rep, t, 0), v_group(rep, t, 0))
            for t in range(4):
                for pr in ("q", "k"):
                    for c in range(2):
                        extend_group((pr, rep, t, c), qk_group(rep, pr, t, c))
            for t in range(NT):
                extend_group(("v", rep, t, 1), v_group(rep, t, 1))
            for t in range(4, NT):
                for pr in ("q", "k"):
                    for c in range(2):
                        extend_group((pr, rep, t, c), qk_group(rep, pr, t, c))

        def input_dmas(rep):
            for j in range(NT):
                nc.sync.dma_start(xT[:, j, :], x_tt[:, j, :])

        def nm_dmas(rep):
            for j in range(NT):
                nc.sync.dma_start(nm[:, j, :], notm_tt[:, j, :])

        def queue_input_dmas(rep):
            # queued (not inline) so the xT overwrite is EMITTED after the
            # previous rep's still-queued projection groups that read xT
            fill.append((("xdma", rep), lambda: input_dmas(rep), True))

        # ---------------- main schedule ----------------
        input_dmas(0)
        nm_dmas(0)
        push_rep_fillers(0)

        for rep in range(repeat):
            boost = 10 if rep == 0 else 0
            pending_norm = []
            if PROBE == "serial":
                while fill:
                    pump(1)
            for p in range(NPAIR):
                # normalize the previous pair's oT (bounce has long landed)
                for pp, pc, pRb in pending_norm:
                    nc.vector.tensor_tensor(
                        oT[:, pp, pc * 512 : (pc + 1) * 512],
                        oT[:, pp, pc * 512 : (pc + 1) * 512],
                        pRb[:],
                        MULT,
                    )
                pending_norm = []
                if p == 4 and rep + 1 < repeat:
                    # queue next rep's input DMA + projections behind this
                    # rep's still-queued tail work
                    queue_input_dmas(rep + 1)
                    push_rep_fillers(rep + 1)
                for pr in ("q", "k"):
                    for cc in range(2):
                        ensure((pr, rep, p, cc))
                ch = p // 4
                # sums rows parked at quadrant-aligned partitions 0/32/64/96
                # (compute engines cannot address odd partition starts)
                SR = p_sr.tile([P, 512], F32, tag="SR", name=f"SR_{rep}_{p}")
                for c in range(2):
                    O = {}
                    Em_prev = None
                    for j in range(NT):
                        L = psL.tile(
                            [P, 2 * 512], F32, tag="L", name=f"L_{rep}_{p}_{c}_{j}"
                        )
                        for h in range(2):
                            r0 = h * DK
                            nc.tensor.matmul(
                                L[:, h * 512 : (h + 1) * 512],
                                KT[r0 : r0 + DK, p, j * P : (j + 1) * P],
                                QT[r0 : r0 + DK, p, c * 512 : (c + 1) * 512],
                                start=True, stop=True,
                            )
                        pump(3 + boost)
                        Em = p_em.tile(
                            [P, 2 * 512], F16, tag="Em", name=f"Em_{rep}_{p}_{c}_{j}"
                        )
                        nc.scalar.activation(Em[:], L[:], Exp, scale=0.125)
                        if PROBE == "nomask":
                            pass  # timing probe only -- wrong results
                        elif PROBE == "mask2d":
                            for h in range(2):
                                nc.vector.tensor_tensor(
                                    Em[:, h * 512 : (h + 1) * 512],
                                    Em[:, h * 512 : (h + 1) * 512],
                                    nm[:, j, c * 512 : (c + 1) * 512],
                                    MULT,
                                )
                        else:
                            # one 3D op, nm broadcast across the head axis
                            nc.vector.tensor_tensor(
                                Em[:].rearrange("p (h q) -> p h q", h=2),
                                Em[:].rearrange("p (h q) -> p h q", h=2),
                                nm[:, j : j + 1, c * 512 : (c + 1) * 512]
                                .broadcast_to((P, 2, 512)),
                                MULT,
                            )
                        # attnV for the PREVIOUS j (lag 1 so PE has slack
                        # between producing Em and consuming it)
                        if j > 0:
                            ensure(("v", rep, j - 1, ch))
                            _emit_attnv(nc, psO, Vg, O, Em_prev, rep, p, c, j - 1)
                        Em_prev = Em
                    pump(2)
                    ensure(("v", rep, NT - 1, ch))
                    _emit_attnv(nc, psO, Vg, O, Em_prev, rep, p, c, NT - 1)
                    # stage O out of PSUM fast (unnormalized) so the
                    # accumulator banks free quickly; ACT collects the two
                    # sums rows into this pair's SR tile for a batched recip
                    for h in range(2):
                        q0 = 32 * (c * 2 + h)
                        nc.scalar.activation(
                            SR[q0 : q0 + 1, :],
                            O[h][DK : DK + 1, :],
                            Copy,
                        )
                        nc.vector.tensor_copy(
                            oT[h * DK : (h + 1) * DK, p, c * 512 : (c + 1) * 512],
                            O[h][0:DK, :],
                        )
                # ---- end of pair p: one 4-lane recip, one DRAM bounce ----
                RR = p_rr.tile([P, 512], F16, tag="RR", name=f"RR_{rep}_{p}")
                # full-tile recip: only the 4 quadrant-head rows are real,
                # the rest is garbage that never leaves RR (DVE free-dim is
                # serial per lane, so 128 rows cost the same as 4)
                with nc.allow_low_precision(reason="f16 softmax recip"):
                    nc.vector.reciprocal(RR[:], SR[:])
                Rd4 = p_dram.tile([4, 512], F16, tag="Rd4", name=f"Rd4_{rep}_{p}")
                nc.gpsimd.dma_start(
                    Rd4[:],
                    RR[:].rearrange("(g r) f -> g r f", r=32)[:, 0, :],
                )
                for c in range(2):
                    Rb = p_rb.tile([P, 512], F16, tag="Rb",
                                   name=f"Rb_{rep}_{p}_{c}")
                    for h in range(2):
                        nc.gpsimd.dma_start(
                            Rb[h * DK : (h + 1) * DK, :],
                            Rd4[c * 2 + h : c * 2 + h + 1, :]
                            .broadcast_to((DK, 512)),
                        )
                    pending_norm.append((p, c, Rb))
                if p == NPAIR - 1 and rep + 1 < repeat:
                    nm_dmas(rep + 1)

            # flush the last deferred normalize, give PE some filler to
            # chew while pair 7's drains complete
            pump(4)
            for pp, pc, pRb in pending_norm:
                nc.vector.tensor_tensor(
                    oT[:, pp, pc * 512 : (pc + 1) * 512],
                    oT[:, pp, pc * 512 : (pc + 1) * 512],
                    pRb[:],
                    MULT,
                )
            pending_norm = []

            # ---- output projection (uses psL buffers; psP stays free
            # for next-rep projection fillers being pumped meanwhile) ----
            for t in range(NT):
                F = psL.tile([P, 2 * 512], F32, tag="L", name=f"F_{rep}_{t}")
                for c in range(2):
                    for j in range(NT):
                        nc.tensor.matmul(
                            F[:, c * 512 : (c + 1) * 512],
                            oT[:, j, t * P : (t + 1) * P],
                            W_sb["o"][:, j, c * 512 : (c + 1) * 512],
                            start=(j == 0), stop=(j == NT - 1),
                        )
                    pump(3)
                ot = p_ot.tile([P, S], F32, tag="ot", name=f"ot_{rep}_{t}")
                nc.vector.tensor_add(ot[:], F[:], bo_sb[:])
                nc.sync.dma_start(out_t[:, t, :], ot[:])

        # drain any leftover fillers (shouldn't be many)
        while fill:
            _k, _th, _last = fill.popleft()
            _th()

    if repeat == 1:
        _cached_nc = nc
    return nc


def _emit_attnv(nc, psO, Vg, O, Em, rep, p, c, j):
    for h in range(2):
        head = 2 * p + h
        if j == 0:
            O[h] = psO.tile(
                [DK + 1, 512], mybir.dt.float32, tag="O",
                name=f"O_{rep}_{p}_{c}_{h}",
            )
        nc.tensor.matmul(
            O[h][:],
            Vg[:, j, head * (DK + 1) : (head + 1) * (DK + 1)],
            Em[:, h * 512 : (h + 1) * 512],
            start=(j == 0), stop=(j == NT - 1),
        )


# ---------------------------------------------------------------------------
# Entry point
# ---------------------------------------------------------------------------
def make_in_maps(x, attn_mask, Wq, bq, Wk, bk, Wv, bv, Wo, bo):
    bqc = np.ascontiguousarray(np.asarray(bq, np.float32).reshape(NT, P).T)
    bo_eff = (
        np.asarray(bv, np.float64) @ np.asarray(Wo, np.float64)
        + np.asarray(bo, np.float64)
    ).astype(np.float32)
    ndt = np.float16
    wqc = np.asarray(Wq, np.float32).astype(ndt)
    wkc = np.asarray(Wk, np.float32).astype(ndt)
    wvc = np.asarray(Wv, np.float32).astype(ndt)
    woc = np.asarray(Wo, np.float32).astype(ndt)
    in_maps = []
    for n in range(N):
        notm = (~np.asarray(attn_mask[n])).astype(ndt)
        in_maps.append(
            {
                "x_t": np.ascontiguousarray(
                    np.asarray(x[n], np.float32).T
                ).astype(ndt),
                "notm_t": np.ascontiguousarray(notm.T),
                "wq": wqc, "wk": wkc, "wv": wvc, "wo": woc,
                "bqc": bqc, "bo_eff": bo_eff,
            }
        )
    return in_maps


def kernel(x, attn_mask, Wq, bq, Wk, bk, Wv, bv, Wo, bo, **_):
    nc = _build()
    in_maps = make_in_maps(x, attn_mask, Wq, bq, Wk, bk, Wv, bv, Wo, bo)
    res = run_bass_kernel_spmd(nc, in_maps, list(range(N)))
    outs = np.stack([np.asarray(res.results[n]["out"]) for n in range(N)], axis=0)
    return outs.astype(np.float32)
